# revision 21
# baseline (speedup 1.0000x reference)
"""Binarized AlexNet on 8 Trainium2 NeuronCores (SPMD, data-parallel over batch).

- Batch 128 sharded 16 imgs/core; weights replicated.
- conv1: 3-pass fp16 (hi/lo split) matmuls over host im2col (error ~1e-6 so the
  bn1 sign decisions match the fp32 reference; bn1 boundary found by bit-exact
  fp32 emulation).
- Binarized activations kept as u in {0,1} fp8: pad(-1) maps to u=0, so conv
  inputs live on zero-padded flat grids; DoubleRow slots are row-shifts /
  channel-group interleaves; sums are exact integers in fp32 PSUM.
- Binarize decisions via host-precomputed per-channel integer thresholds that
  exactly emulate the reference's fp32 bn rounding (validated: 0 mismatches
  over all (channel, attainable-integer) pairs). Half-offset -> never ties.
- All odd-row taps run as DoubleRow with a zero weight row (half the col cost
  of a single-row fp8 matmul); the phantom slot reads the row above, in-bounds.
- Small weights are packed host-side into a handful of wide buffers (one DMA
  each on the SP HWDGE queue); im2col cols arrive as one DMA per image.
- fc1/fc2: activation-stationary fp8 DR matmuls, weight-tile-outer loop with 8
  resident PSUM chunk accumulators, streaming whole [128,2,4096] weight tiles
  (one DMA per tile); thresholds via rank-1 (K=2 fp16) matmuls; PE transposes
  between layers; bn7 folded into ACT Relu; fc3 in fp16 streamed 4 tiles/DMA.
"""
import dataclasses
import numpy as np
import ml_dtypes

import bass_rust
import concourse.bass as bass
import concourse.tile as tile
from concourse import mybir
from concourse.bass_utils import run_bass_kernel_spmd
from concourse.tile_rust import add_dep_helper

FP8 = ml_dtypes.float8_e4m3
BF16 = ml_dtypes.bfloat16

EPS = np.float32(1e-5)
NCORES = 8
DR = mybir.MatmulPerfMode.DoubleRow

# grids: (stride, base, tile-size, out-flat-count)
G2, B2, S2, N2 = 32, 66, 1024, 27 * 32          # conv2 in-grid (27x27 interior)
G3, B3, S3, N3 = 15, 16, 240, 13 * 15           # conv3 in-grid (13x13)
G4, B4, S4, N4 = 16, 17, 256, 13 * 16           # conv4/5 in-grid (13x13)


def apv(base, offset_add, dims):
    return dataclasses.replace(base, offset=base.offset + offset_add, ap=dims)


# ======================= host-side preparation =======================

def _f32(x):
    return np.asarray(x, dtype=np.float32)


def bn_scale(bnp):
    g, b, m, v = [_f32(a) for a in bnp]
    s = (g / np.sqrt(v + EPS)).astype(np.float32)
    return s, b, m


def exact_bn1_threshold(bnp):
    """Per-channel minimal fp32 x with ((x-m)*s)+b >= 0 under fp32 rounding."""
    s, b, m = bn_scale(bnp)
    assert np.all(s > 0)
    C = len(s)

    def dec(x):
        with np.errstate(over='ignore', invalid='ignore'):
            return ((x - m).astype(np.float32) * s).astype(np.float32) + b >= 0

    def key(f):  # monotone uint32 encoding of fp32 order
        i = f.view(np.uint32).astype(np.uint32)
        neg = (i >> 31).astype(bool)
        return np.where(neg, ~i, i | np.uint32(0x80000000)).astype(np.uint32)

    def unkey(k):
        k = k.astype(np.uint32)
        hi = (k >> 31).astype(bool)
        return np.where(hi, k & np.uint32(0x7FFFFFFF), ~k).astype(np.uint32).view(np.float32)

    lo = np.full(C, -3.0e38, np.float32)
    hi = np.full(C, 3.0e38, np.float32)
    always = dec(lo)
    never = ~dec(hi)
    klo, khi = key(lo).astype(np.uint64), key(hi).astype(np.uint64)
    for _ in range(40):
        kmid = ((klo + khi) // 2).astype(np.uint64)
        d = dec(unkey(kmid.astype(np.uint32)))
        khi = np.where(d, kmid, khi)
        klo = np.where(d, klo, kmid)
    t = unkey(khi.astype(np.uint32))
    t = np.where(always, np.float32(-3.0e38), t)
    t = np.where(never, np.float32(3.0e38), t)
    chk = dec(t) | never
    assert np.all(chk), "bn1 threshold search failed"
    return t.astype(np.float32)


def integer_threshold(bnp, rmax):
    """Min even integer E in [-rmax,rmax] with fp32-bn(E) >= 0, per channel."""
    s, b, m = bn_scale(bnp)
    assert np.all(s > 0)
    C = len(s)
    T = np.zeros(C, np.float64)
    E = np.arange(-rmax, rmax + 1, 2, dtype=np.float32)
    for c0 in range(0, C, 512):
        c1 = min(C, c0 + 512)
        pre = ((E[None, :] - m[c0:c1, None]).astype(np.float32)
               * s[c0:c1, None]).astype(np.float32) + b[c0:c1, None]
        d = pre >= 0
        anyd = d.any(axis=1)
        t = E[np.argmax(d, axis=1)].astype(np.float64)
        t = np.where(anyd, t, rmax + 2)
        t = np.where(d[:, 0], -rmax - 2, t)
        T[c0:c1] = t
    return T


def binarize_w(w):
    return np.where(_f32(w) >= 0, 1.0, -1.0).astype(np.float32)


def prep_host(inputs):
    P = {}
    x = _f32(inputs['x'])
    Bfull = x.shape[0]

    # ---- conv1 im2col (fp16 hi/lo), packed hi(3ct)+lo(3ct) per image ----
    xp = np.zeros((Bfull, 3, 228, 228), np.float32)
    xp[:, :, 2:226, 2:226] = x
    xh = xp.astype(np.float16)
    xl = (xp - xh.astype(np.float32)).astype(np.float16)

    def im2col(a):
        s = a.strides
        v = np.lib.stride_tricks.as_strided(
            a, (Bfull, 3, 11, 11, 55, 55),
            (s[0], s[1], s[2], s[3], s[2] * 4, s[3] * 4))
        return v.reshape(Bfull, 3, 121, 3025)

    P['colhl'] = np.concatenate([im2col(xh), im2col(xl)], axis=1).copy()  # [B,6,121,3025]

    w1 = _f32(inputs['conv1_w']).reshape(96, 3, 121)
    w1h = w1.astype(np.float16)
    w1l = (w1 - w1h.astype(np.float32)).astype(np.float16)
    # wp121 [121, 576]: hi0|hi1|hi2|lo0|lo1|lo2, each [121,96]
    P['wp121'] = np.concatenate(
        [w1h.transpose(1, 2, 0)[ct] for ct in range(3)]
        + [w1l.transpose(1, 2, 0)[ct] for ct in range(3)], axis=1).astype(np.float16)

    t1x = exact_bn1_threshold(inputs['bn1']).reshape(96, 1)

    w2 = binarize_w(inputs['bconv2_w'])
    w3 = binarize_w(inputs['bconv3_w'])
    w4 = binarize_w(inputs['bconv4_w'])
    w5 = binarize_w(inputs['bconv5_w'])
    w6 = binarize_w(inputs['blin1_w'])
    w7 = binarize_w(inputs['blin2_w'])

    # conv2: all-DR layout [2 mt][5 kx][3 kg][96, 2, 128]; kg=2 holds the ky=4
    # row in slot j=1 with a zero row in slot j=0 (rhs base shifted one row up)
    w2p = np.zeros((2, 5, 3, 96, 2, 128), np.float32)
    for mt in range(2):
        for kx in range(5):
            for kg in range(2):
                for j in range(2):
                    w2p[mt, kx, kg, :, j, :] = w2[mt * 128:(mt + 1) * 128, :, 2 * kg + j, kx].T
            w2p[mt, kx, 2, :, 1, :] = w2[mt * 128:(mt + 1) * 128, :, 4, kx].T
    P['wp96'] = w2p.reshape(2 * 5 * 3, 96, 256).transpose(1, 0, 2).reshape(96, -1).astype(FP8)
    K2 = w2.reshape(256, -1).sum(1)
    T2 = integer_threshold(inputs['bn2'], 2400)
    thr2u = ((T2 + K2) / 2 - 0.5).astype(np.float32).reshape(2, 128).T.copy()  # [128,2]

    w3dr = np.zeros((3, 3, 3, 128, 2, 128), np.float32)
    for mt in range(3):
        for ky in range(3):
            for kx in range(3):
                for j in range(2):
                    w3dr[mt, ky, kx, :, j, :] = w3[mt * 128:(mt + 1) * 128, 128 * j:128 * (j + 1), ky, kx].T
    K3 = w3.reshape(384, -1).sum(1)
    T3 = integer_threshold(inputs['bn3'], 2304)
    thr3u = ((T3 + K3) / 2 - 0.5).astype(np.float32).reshape(3, 128).T.copy()  # [128,3]

    def pack45(w, nmt):
        wa = np.zeros((nmt, 3, 3, 128, 2, 128), np.float32)
        wbd = np.zeros((nmt, 3, 128, 2, 128), np.float32)
        wbs = np.zeros((nmt, 3, 128, 2, 128), np.float32)   # j0 zero, j1 real
        for mt in range(nmt):
            for kx in range(3):
                for ky in range(3):
                    for j in range(2):
                        wa[mt, ky, kx, :, j, :] = w[mt * 128:(mt + 1) * 128, 128 * j:128 * (j + 1), ky, kx].T
                for j in range(2):
                    wbd[mt, kx, :, j, :] = w[mt * 128:(mt + 1) * 128, 256:384, j, kx].T
                wbs[mt, kx, :, 1, :] = w[mt * 128:(mt + 1) * 128, 256:384, 2, kx].T
        return wa, wbd, wbs

    w4a, w4bd, w4bs = pack45(w4, 3)
    K4 = w4.reshape(384, -1).sum(1)
    T4 = integer_threshold(inputs['bn4'], 3456)
    thr4u = ((T4 + K4) / 2 - 0.5).astype(np.float32).reshape(3, 128).T.copy()

    w5a, w5bd, w5bs = pack45(w5, 2)
    K5 = w5.reshape(256, -1).sum(1)
    T5 = integer_threshold(inputs['bn5'], 3456)
    thr5u = ((T5 + K5) / 2 - 0.5).astype(np.float32).reshape(2, 128).T.copy()

    # wp128 [128, 26112] fp8: w3dr(27) | w4a(27) | w4bd(9) | w4bs(9) | w5a(18)
    #                         | w5bd(6) | w5bs(6), each tile 256 cols
    tiles128 = []
    for mt in range(3):
        for ky in range(3):
            for kx in range(3):
                tiles128.append(w3dr[mt, ky, kx])
    for arr, nmt in ((w4a, 3), (w4bd, 3), (w4bs, 3), (w5a, 2), (w5bd, 2), (w5bs, 2)):
        if arr.ndim == 6:
            for mt in range(nmt):
                for ky in range(3):
                    for kx in range(3):
                        tiles128.append(arr[mt, ky, kx])
        else:
            for mt in range(nmt):
                for kx in range(3):
                    tiles128.append(arr[mt, kx])
    P['wp128'] = np.concatenate(
        [t.reshape(128, 256) for t in tiles128], axis=1).astype(FP8)

    # wpf32 [128, 96]: t1x(1) thr2u(2) thr3u(3) thr4u(3) thr5u(2) sc7(32)
    #                  bi7(32) id16f(16 on rows 0:16)
    s7, b7, m7 = bn_scale(inputs['bn7'])
    K7 = w7.sum(1)
    sc7 = (2.0 * s7).astype(np.float32).reshape(32, 128).T.copy()      # [128,32]
    bi7 = (((-K7 - m7) * s7) + b7).astype(np.float32).reshape(32, 128).T.copy()
    wpf32 = np.zeros((128, 96), np.float32)
    wpf32[0:96, 0:1] = t1x
    wpf32[:, 1:3] = thr2u
    wpf32[:, 3:6] = thr3u
    wpf32[:, 6:9] = thr4u
    wpf32[:, 9:11] = thr5u
    wpf32[:, 11:43] = sc7
    wpf32[:, 43:75] = bi7
    wpf32[0:16, 75:91] = np.eye(16, dtype=np.float32)
    P['wpf32'] = wpf32

    # fc1 rhs: w6t[s][p, j, n] = w6[n, (p+128j)*36 + s]
    w6r = np.zeros((36, 128, 2, 4096), np.float32)
    for j in range(2):
        for s in range(36):
            cols = (np.arange(128) + 128 * j) * 36 + s
            w6r[s, :, j, :] = w6[:, cols].T
    P['w6t_full'] = w6r.astype(FP8)
    K6 = w6.sum(1)
    T6 = integer_threshold(inputs['bn6'], 9216)
    t6u = (T6 + K6) / 2 - 0.5
    t6hi = np.floor(t6u / 16) * 16
    t6lo = t6u - t6hi
    thr6 = np.stack([-t6hi, -t6lo]).astype(np.float16)   # [2, 4096]
    assert np.array_equal(thr6.astype(np.float64).sum(0), -t6u)
    P['thr6_full'] = thr6

    w7r = np.zeros((16, 128, 2, 4096), np.float32)
    for kc in range(16):
        for j in range(2):
            w7r[kc, :, j, :] = w7[:, 256 * kc + 128 * j: 256 * kc + 128 * (j + 1)].T
    P['w7t_full'] = w7r.astype(FP8)

    # fc2 epilogue: relu(sc7*S + bi7) == sc7 * relu(S + bi7/sc7) (sc7 > 0);
    # the sc7 factor is folded into the fc3 weights host-side.
    s7, b7, m7 = bn_scale(inputs['bn7'])
    assert np.all(s7 > 0)
    K7 = w7.sum(1)
    sc7f = (2.0 * s7).astype(np.float64)
    bi7f = ((-K7 - m7) * s7 + b7).astype(np.float64)
    cf = bi7f / sc7f
    chi = cf.astype(np.float16)
    clo = (cf - chi.astype(np.float64)).astype(np.float16)
    P['crow_full'] = np.stack([chi, clo])                # [2, 4096]

    w8 = _f32(inputs['lin3_w'])
    w8s = np.zeros((32, 128, 1000), np.float32)
    for kc in range(32):
        cols = np.arange(128 * kc, 128 * (kc + 1))
        w8s[kc] = (w8[:, cols].T * sc7f[cols, None]).astype(np.float32)
    P['w8s_full'] = w8s.astype(np.float16)
    b8 = _f32(inputs['lin3_b'])
    b8d = b8.astype(np.float64) / NCORES                 # each core adds b8/8
    b8hi = b8d.astype(np.float16)
    b8p = np.stack([b8hi, (b8d - b8hi.astype(np.float64)).astype(np.float16)])
    P['b8d'] = b8p.astype(np.float16)                    # [2, 1000]

    P['id128'] = np.eye(128).astype(np.float16)
    return P


# mode: 'img' = per-image batch slice, 'core' = per-core slice, None = shared
IN_SPECS = [
    ('colhl', (6, 121, 3025), mybir.dt.float16, 'img'),
    ('wp121', (121, 576), mybir.dt.float16, None),
    ('wp96', (96, 7680), mybir.dt.float8e4, None),
    ('wp128', (128, 26112), mybir.dt.float8e4, None),
    ('wpf32', (128, 96), mybir.dt.float32, None),
    ('wpf16', (2, 2024), mybir.dt.float16, 'core'),   # thr6_c | crow_c | b8d
    ('id128', (128, 128), mybir.dt.float16, None),
    ('cmask', (128, 8), mybir.dt.float32, 'core'),    # one-hot core id
    ('w6tp', (36, 128, 2, 512), mybir.dt.float8e4, 'core'),
    ('w7tp', (16, 128, 2, 512), mybir.dt.float8e4, 'core'),
    ('w8sp', (4, 128, 1000), mybir.dt.float16, 'core'),
]


def build_module(B, dbg=False):
    nc = bass.Bass("TRN2", target_bir_lowering=False, debug=False,
                   num_devices=NCORES)
    ins = {}
    for name, shp, dt, mode in IN_SPECS:
        shape = (B,) + shp if mode == 'img' else shp
        ins[name] = nc.dram_tensor(name, list(shape), dt, kind="ExternalInput").ap()

    out = nc.dram_tensor("out", [B, 1000], mybir.dt.float32, kind="ExternalOutput").ap()

    # cross-core exchange buffers (AllReduce with zero-padded per-core slots
    # emulates AllGather, whose direct form misroutes under this runtime)
    RG = [list(range(NCORES))]
    ag5_in = [nc.dram_tensor(f"ag5_in{h}", [128, 4608], mybir.dt.float8e4,
                             kind="Internal").ap() for h in range(2)]
    ag5_out = [nc.dram_tensor(f"ag5_out{h}", [128, 4608], mybir.dt.float8e4,
                              kind="Internal", addr_space="Shared").ap()
               for h in range(2)]
    ag6_in = nc.dram_tensor("ag6_in", [128, 4096], mybir.dt.float8e4,
                            kind="Internal").ap()
    ag6_out = nc.dram_tensor("ag6_out", [128, 4096], mybir.dt.float8e4,
                             kind="Internal", addr_space="Shared").ap()
    rs_in = nc.dram_tensor("rs_in", [128, 1000], mybir.dt.float32,
                           kind="Internal").ap()
    rs_out = nc.dram_tensor("rs_out", [B, 1000], mybir.dt.float32,
                            kind="Internal").ap()
    dbg_outs = {}
    if dbg:
        for name, shape, dt in [
            ('d_u1', [B, 96, S2], mybir.dt.float8e4),
            ('d_u2', [B, 128, 2, S3], mybir.dt.float8e4),
            ('d_u3a', [B, 128, 2, S4], mybir.dt.float8e4),
            ('d_u3b', [B, 128, S4], mybir.dt.float8e4),
            ('d_u4a', [B, 128, 2, S4], mybir.dt.float8e4),
            ('d_u4b', [B, 128, S4], mybir.dt.float8e4),
            ('d_t5', [128, 2, 36, B], mybir.dt.float8e4),
            ('d_t6', [128, 16, 2, B], mybir.dt.float8e4),
        ]:
            dbg_outs[name] = nc.dram_tensor(name, shape, dt, kind="ExternalOutput").ap()

    dma_handles = []
    tail_extra = []

    def ldma(dst, src):
        h = nc.sync.dma_start(dst, src)
        dma_handles.append(h)
        return h

    def odma(dst, src):
        h = nc.sync.dma_start(dst, src)
        dma_handles.append(h)
        return h

    with tile.TileContext(nc) as tc:
        with tc.tile_pool(name="wpool", bufs=1) as wp, \
             tc.tile_pool(name="cpool", bufs=1) as cp:
            colp_cm = tc.tile_pool(name="col", bufs=2)
            colp = colp_cm.__enter__()

            # ---------- packed constant tiles (one DMA each) ----------
            # conv1-critical packs first so image 0's col DMA isn't queued
            # behind the big conv2+ packs
            pk121 = wp.tile([121, 576], mybir.dt.float16, tag="pk121")
            ldma(pk121[:], ins['wp121'][:])
            pkf32 = wp.tile([128, 96], mybir.dt.float32, tag="pkf32")
            ldma(pkf32[:], ins['wpf32'][:])
            col0 = colp.tile([121, 6 * 3025], mybir.dt.float16, tag="colhl")
            for n0 in range(0, 3025, 512):
                n1 = min(3025, n0 + 512)
                ldma(apv(col0[:], n0, [[6 * 3025, 121], [3025, 6], [1, n1 - n0]]),
                     apv(ins['colhl'][0], n0, [[3025, 121], [121 * 3025, 6], [1, n1 - n0]]))
            pk96 = wp.tile([96, 7680], mybir.dt.float8e4, tag="pk96")
            ldma(pk96[:], ins['wp96'][:])
            pk128 = wp.tile([128, 26112], mybir.dt.float8e4, tag="pk128")
            ldma(pk128[:], ins['wp128'][:])
            pkf16 = wp.tile([2, 2024], mybir.dt.float16, tag="pkf16")
            ldma(pkf16[:], ins['wpf16'][:])
            t_id128 = wp.tile([128, 128], mybir.dt.float16, tag="id128")
            ldma(t_id128[:], ins['id128'][:])
            t_cmask = wp.tile([128, 8], mybir.dt.float32, tag="cmask")
            ldma(t_cmask[:], ins['cmask'][:])

            # ---------- views into the packs ----------
            W = {}
            for ct in range(3):
                W[f'w1_hi{ct}'] = pk121[:, ct * 96:(ct + 1) * 96]
                W[f'w1_lo{ct}'] = pk121[:, (3 + ct) * 96:(4 + ct) * 96]
            for mt in range(2):
                for kx in range(5):
                    for kg in range(3):
                        c0 = (mt * 15 + kx * 3 + kg) * 256
                        W[f'w2dr{mt}{kx}{kg}'] = pk96[:, c0:c0 + 256].rearrange(
                            "p (j n) -> p j n", j=2)
            ti = 0
            for mt in range(3):
                for ky in range(3):
                    for kx in range(3):
                        W[f'w3dr{mt}{ky}{kx}'] = pk128[:, ti * 256:(ti + 1) * 256].rearrange(
                            "p (j n) -> p j n", j=2)
                        ti += 1
            for nm, nmt in (('w4', 3), ('w5', 2)):
                for mt in range(nmt):
                    for ky in range(3):
                        for kx in range(3):
                            W[f'{nm}a{mt}{ky}{kx}'] = pk128[:, ti * 256:(ti + 1) * 256].rearrange(
                                "p (j n) -> p j n", j=2)
                            ti += 1
                for grp in ('bd', 'bs'):
                    for mt in range(nmt):
                        for kx in range(3):
                            W[f'{nm}{grp}{mt}{kx}'] = pk128[:, ti * 256:(ti + 1) * 256].rearrange(
                                "p (j n) -> p j n", j=2)
                            ti += 1
            assert ti * 256 == 26112
            W['t1x'] = pkf32[0:96, 0:1]
            W['thr2u'] = pkf32[:, 1:3]
            W['thr3u'] = pkf32[:, 3:6]
            W['thr4u'] = pkf32[:, 6:9]
            W['thr5u'] = pkf32[:, 9:11]
            W['thr6'] = pkf16[:, 0:512]
            W['crow'] = pkf16[:, 512:1024]
            W['b8d'] = pkf16[:, 1024:2024]
            W['id128'] = t_id128[:]
            W['cmask'] = t_cmask[:]

            t_ones = wp.tile([2, 128], mybir.dt.float16, tag="ones")
            nc.vector.memset(t_ones[:], 1.0)

            last_out = {}   # proc -> 1-cell AP of that engine's latest evac output

            # DVE/ACT intro: first-touch the packed tiles' DMA queues
            for k, key in enumerate(('t1x', 'thr2u', 'thr3u', 'thr4u', 'thr5u',
                                     'cmask')):
                t = W[key]
                scr = wp.tile([128, 1], mybir.dt.float32, tag=f"scr{k}")
                nc.vector.tensor_copy(scr[0:t.shape[0], 0:1], t[:, 0:1])
            for k, key in enumerate(('thr6', 'crow', 'b8d')):
                t = W[key]
                scr = wp.tile([128, 1], mybir.dt.float32, tag=f"scrA{k}")
                nc.scalar.copy(scr[0:t.shape[0], 0:1], t[:, 0:1])

            # fc1 lhsT source, split by image half so the first half's
            # cross-core exchange overlaps the second half's conv
            Bh = B // 2
            t5h = []
            for h in range(2):
                t5x = cp.tile([128, 2 * 36 * Bh], mybir.dt.float8e4, tag=f"t5{h}")
                nc.vector.memset(t5x[:], 0.0)
                t5h.append(t5x)
            # zero-padded AllReduce staging (zeroed early, off the critical path)
            zb5 = []
            for h in range(2):
                zb5x = cp.tile([128, 4608], mybir.dt.float8e4, tag=f"zb5{h}")
                nc.vector.memset(zb5x[:], 0.0)
                zb5.append(zb5x)
            zb6 = cp.tile([128, 4096], mybir.dt.float8e4, tag="zb6")
            nc.vector.memset(zb6[:], 0.0)


            # ================= conv phase =================
            with tc.tile_pool(name="act", bufs=5) as ap_, \
                 tc.tile_pool(name="blk", bufs=2) as bp, \
                 tc.tile_pool(name="stage", bufs=1) as stp, \
                 tc.tile_pool(name="ps1", bufs=2, space="PSUM") as ps1, \
                 tc.tile_pool(name="ps2", bufs=1, space="PSUM") as ps2, \
                 tc.tile_pool(name="ps3", bufs=4, space="PSUM") as ps3:

                def emit_t5_exchange(h):
                    # slot core k's images into the zero-padded buffer, then
                    # sum-exchange (slots are disjoint, so add == gather)
                    for k in range(NCORES):
                        nc.vector.tensor_scalar(
                            apv(zb5[h][:], Bh * k,
                                [[4608, 128], [2304, 2], [Bh * NCORES, 36], [1, Bh]]),
                            apv(t5h[h][:], 0,
                                [[2 * 36 * Bh, 128], [36 * Bh, 2], [Bh, 36], [1, Bh]]),
                            W['cmask'][:, k:k + 1], None, mybir.AluOpType.mult)
                    ldma(ag5_in[h][:], zb5[h][:])
                    nc.gpsimd.collective_compute(
                        "AllReduce", mybir.AluOpType.add,
                        ins=[ag5_in[h][:]], outs=[ag5_out[h][:]],
                        replica_groups=RG)

                NB = 4
                NP = NB // 2          # image pairs per block
                PS2b = NB * 2 * S3    # u2b per-partition row length
                PS4ab = NB * 2 * S4
                PS4bb = NB * S4
                for i0 in range(0, B, NB):
                    blk = list(range(i0, min(B, i0 + NB)))
                    # block activation tiles; per-image planes side by side so
                    # conv3/4/5 matmuls can stream 2 images per instruction
                    u2b = bp.tile([128, PS2b], mybir.dt.float8e4, tag="u2b")
                    u3ab = bp.tile([128, PS4ab], mybir.dt.float8e4, tag="u3ab")
                    u3bb = bp.tile([128, PS4bb], mybir.dt.float8e4, tag="u3bb")
                    u4ab = bp.tile([128, PS4ab], mybir.dt.float8e4, tag="u4ab")
                    u4bb = bp.tile([128, PS4bb], mybir.dt.float8e4, tag="u4bb")
                    for i in blk:
                        # ---- conv1 (cols: one DMA per image) ----
                        if i == 0:
                            colhl = col0
                        else:
                            colhl = colp.tile([121, 6 * 3025], mybir.dt.float16, tag="colhl")
                            ldma(colhl[:],
                                 apv(ins['colhl'][i], 0,
                                     [[3025, 121], [121 * 3025, 6], [1, 3025]]))
                        colh = colhl[:, 0:3 * 3025]
                        coll = colhl[:, 3 * 3025:6 * 3025]

                        c1f = stp.tile([96, 3025], mybir.dt.float32, tag="c1f")
                        passes = [('w1_hi', colh), ('w1_hi', coll), ('w1_lo', colh)]
                        for n0 in range(0, 3025, 512):
                            n1 = min(3025, n0 + 512)
                            pc = ps1.tile([96, 512], mybir.dt.float32, tag="pc1")
                            cnt = 0
                            for ct in range(3):
                                for wk, colt in passes:
                                    cnt += 1
                                    nc.tensor.matmul(
                                        pc[:, 0:n1 - n0], W[wk + str(ct)],
                                        apv(colt, ct * 3025 + n0,
                                            [[6 * 3025, 121], [1, n1 - n0]]),
                                        start=(cnt == 1), stop=(cnt == 9))
                            last_out['ACT'] = nc.scalar.copy(c1f[:, n0:n1], pc[:, 0:n1 - n0])

                        # pool1: 55x55 -> 27x27
                        pm1 = stp.tile([96, 55 * 27], mybir.dt.float32, tag="pm1")
                        d = [[3025, 96], [55, 55], [2, 27]]
                        o = [[55 * 27, 96], [27, 55], [1, 27]]
                        nc.vector.tensor_max(apv(pm1[:], 0, o), apv(c1f[:], 0, d), apv(c1f[:], 1, d))
                        nc.vector.tensor_max(apv(pm1[:], 0, o), apv(pm1[:], 0, o), apv(c1f[:], 2, d))
                        po1 = stp.tile([96, 729], mybir.dt.float32, tag="po1")
                        d2 = [[55 * 27, 96], [54, 27], [1, 27]]
                        o2 = [[729, 96], [27, 27], [1, 27]]
                        nc.vector.tensor_max(apv(po1[:], 0, o2), apv(pm1[:], 0, d2), apv(pm1[:], 27, d2))
                        nc.vector.tensor_max(apv(po1[:], 0, o2), apv(po1[:], 0, o2), apv(pm1[:], 54, d2))

                        # u1 border zeros: head [0,B2), row gaps, tail
                        u1 = ap_.tile([96, S2], mybir.dt.float8e4, tag="u1")
                        nc.vector.memset(u1[:, 0:B2], 0.0)
                        nc.vector.memset(
                            apv(u1[:], B2 + 27, [[S2, 96], [G2, 26], [1, 5]]), 0.0)
                        nc.vector.memset(u1[:, B2 + 26 * G2 + 27:S2], 0.0)
                        last_out['DVE'] = nc.vector.tensor_scalar(
                            apv(u1[:], B2, [[S2, 96], [G2, 27], [1, 27]]),
                            apv(po1[:], 0, [[729, 96], [27, 27], [1, 27]]),
                            W['t1x'][:, 0:1], None,
                            mybir.AluOpType.is_ge)
                        if dbg:
                            odma(dbg_outs['d_u1'][i], u1[:])

                        # ---- conv2 (all taps DoubleRow; kg=2 is zero-padded) ----
                        # u2 border zeros: per-plane head / row gaps / tail
                        cb2 = (i - i0) * 2 * S3
                        nc.vector.memset(
                            apv(u2b[:], cb2, [[PS2b, 128], [S3, 2], [1, B3]]), 0.0)
                        nc.vector.memset(
                            apv(u2b[:], cb2 + B3 + 13,
                                [[PS2b, 128], [S3, 2], [G3, 13], [1, 2]]), 0.0)
                        nc.vector.memset(
                            apv(u2b[:], cb2 + B3 + 13 * G3,
                                [[PS2b, 128], [S3, 2], [1, 29]]), 0.0)
                        for mt in range(2):
                            # packed 729-col output: rows 0-17 at psum cols
                            # 0..486 (bank0), rows 18-26 at 512..755 (bank1)
                            p2 = ps2.tile([128, 756], mybir.dt.float32, tag="p2")
                            nmm = 0
                            tot = 5 * 3 * 2
                            for kx in range(5):
                                dx = kx - 2
                                for kg in range(3):
                                    # kg 0,1: rows (2kg, 2kg+1); kg 2: j0 zero
                                    # row (reads ky=3 row), j1 real ky=4 row
                                    if kg < 2:
                                        off = B2 + (2 * kg - 2) * G2 + dx
                                    else:
                                        off = B2 + 1 * G2 + dx
                                    for r0, nr, p0 in ((0, 18, 0), (18, 9, 512)):
                                        nmm += 1
                                        nc.tensor.matmul(
                                            apv(p2[:], p0, [[756, 128], [27, nr], [1, 27]]),
                                            W[f'w2dr{mt}{kx}{kg}'],
                                            apv(u1[:], off + r0 * G2,
                                                [[S2, 96], [G2, 2], [G2, nr], [1, 27]]),
                                            start=(kx == 0 and kg == 0),
                                            stop=(nmm >= tot - 1), perf_mode=DR)
                            c2f = stp.tile([128, 756], mybir.dt.float32, tag="c2f")
                            last_out['ACT'] = nc.scalar.copy(c2f[:], p2[:])
                            pm2 = stp.tile([128, 27 * 13], mybir.dt.float32, tag="pm2")
                            o = [[27 * 13, 128], [13, 27], [1, 13]]
                            for r0, nr, p0 in ((0, 18, 0), (18, 9, 512)):
                                d = [[756, 128], [27, nr], [2, 13]]
                                ov = [[27 * 13, 128], [13, nr], [1, 13]]
                                nc.vector.tensor_max(
                                    apv(pm2[:], r0 * 13, ov),
                                    apv(c2f[:], p0, d), apv(c2f[:], p0 + 1, d))
                                nc.vector.tensor_max(
                                    apv(pm2[:], r0 * 13, ov),
                                    apv(pm2[:], r0 * 13, ov), apv(c2f[:], p0 + 2, d))
                            po2 = stp.tile([128, 169], mybir.dt.float32, tag="po2")
                            d2 = [[27 * 13, 128], [26, 13], [1, 13]]
                            o2 = [[169, 128], [13, 13], [1, 13]]
                            nc.vector.tensor_max(apv(po2[:], 0, o2), apv(pm2[:], 0, d2), apv(pm2[:], 13, d2))
                            nc.vector.tensor_max(apv(po2[:], 0, o2), apv(po2[:], 0, o2), apv(pm2[:], 26, d2))
                            last_out['DVE'] = nc.vector.tensor_scalar(
                                apv(u2b[:], cb2 + mt * S3 + B3,
                                    [[PS2b, 128], [G3, 13], [1, 13]]),
                                apv(po2[:], 0, o2), W['thr2u'][:, mt:mt + 1], None,
                                mybir.AluOpType.is_ge)
                        if dbg:
                            odma(dbg_outs['d_u2'][i],
                                 apv(u2b[:], cb2, [[PS2b, 128], [S3, 2], [1, S3]]))

                    # ---- conv3: 2 images per matmul (4-dim rhs AP) ----
                    for i in blk:
                        cba = (i - i0) * 2 * S4
                        cbb = (i - i0) * S4
                        nc.vector.memset(
                            apv(u3ab[:], cba, [[PS4ab, 128], [S4, 2], [1, B4]]), 0.0)
                        nc.vector.memset(
                            apv(u3ab[:], cba + B4 + 13,
                                [[PS4ab, 128], [S4, 2], [G4, 13], [1, 3]]), 0.0)
                        nc.vector.memset(
                            apv(u3ab[:], cba + B4 + 13 * G4,
                                [[PS4ab, 128], [S4, 2], [1, 31]]), 0.0)
                        nc.vector.memset(
                            apv(u3bb[:], cbb, [[PS4bb, 128], [1, B4]]), 0.0)
                        nc.vector.memset(
                            apv(u3bb[:], cbb + B4 + 13,
                                [[PS4bb, 128], [G4, 13], [1, 3]]), 0.0)
                        nc.vector.memset(
                            apv(u3bb[:], cbb + B4 + 13 * G4,
                                [[PS4bb, 128], [1, S4 - B4 - 13 * G4]]), 0.0)
                    for mt in range(3):
                        for p in range(NP):
                            pp = ps3.tile([128, 512], mybir.dt.float32, tag="p45")
                            for ky in range(3):
                                for kx in range(3):
                                    off = B3 + (ky - 1) * G3 + (kx - 1)
                                    nc.tensor.matmul(
                                        apv(pp[:], 0, [[512, 128], [256, 2], [1, N3]]),
                                        W[f'w3dr{mt}{ky}{kx}'],
                                        apv(u2b[:], (2 * p) * 2 * S3 + off,
                                            [[PS2b, 128], [S3, 2], [2 * S3, 2], [1, N3]]),
                                        start=(ky == 0 and kx == 0),
                                        stop=(ky == 2 and kx == 2),
                                        perf_mode=DR)
                            for q in range(2):
                                i2 = 2 * p + q
                                srcp = apv(pp[:], q * 256, [[512, 128], [G3, 13], [1, 13]])
                                if mt < 2:
                                    dstu = apv(u3ab[:], i2 * 2 * S4 + mt * S4 + B4,
                                               [[PS4ab, 128], [G4, 13], [1, 13]])
                                else:
                                    dstu = apv(u3bb[:], i2 * S4 + B4,
                                               [[PS4bb, 128], [G4, 13], [1, 13]])
                                last_out['DVE'] = nc.vector.tensor_scalar(
                                    dstu, srcp, W['thr3u'][:, mt:mt + 1],
                                    None, mybir.AluOpType.is_ge)
                    if dbg:
                        for i in blk:
                            cba = (i - i0) * 2 * S4
                            cbb = (i - i0) * S4
                            odma(dbg_outs['d_u3a'][i],
                                 apv(u3ab[:], cba, [[PS4ab, 128], [S4, 2], [1, S4]]))
                            odma(dbg_outs['d_u3b'][i],
                                 apv(u3bb[:], cbb, [[PS4bb, 128], [1, S4]]))

                    # ---- conv4 / conv5: 2 images per matmul ----
                    for i in blk:
                        cba = (i - i0) * 2 * S4
                        cbb = (i - i0) * S4
                        nc.vector.memset(
                            apv(u4ab[:], cba, [[PS4ab, 128], [S4, 2], [1, B4]]), 0.0)
                        nc.vector.memset(
                            apv(u4ab[:], cba + B4 + 13,
                                [[PS4ab, 128], [S4, 2], [G4, 13], [1, 3]]), 0.0)
                        nc.vector.memset(
                            apv(u4ab[:], cba + B4 + 13 * G4,
                                [[PS4ab, 128], [S4, 2], [1, 31]]), 0.0)
                        nc.vector.memset(
                            apv(u4bb[:], cbb, [[PS4bb, 128], [1, B4]]), 0.0)
                        nc.vector.memset(
                            apv(u4bb[:], cbb + B4 + 13,
                                [[PS4bb, 128], [G4, 13], [1, 3]]), 0.0)
                        nc.vector.memset(
                            apv(u4bb[:], cbb + B4 + 13 * G4,
                                [[PS4bb, 128], [1, S4 - B4 - 13 * G4]]), 0.0)

                    def conv45blk(nm, nmt, uab, ubb, sink):
                        for mt in range(nmt):
                            pps = []
                            for p in range(NP):
                                ppf = ps3.tile([128, 512], mybir.dt.float32, tag="p45")
                                pps.append(ppf)
                            for ky in range(3):
                                for kx in range(3):
                                    off = B4 + (ky - 1) * G4 + (kx - 1)
                                    for p in range(NP):
                                        nc.tensor.matmul(
                                            apv(pps[p][:], 0, [[512, 128], [256, 2], [1, N4]]),
                                            W[f'{nm}a{mt}{ky}{kx}'],
                                            apv(uab[:], (2 * p) * 2 * S4 + off,
                                                [[PS4ab, 128], [S4, 2], [2 * S4, 2], [1, N4]]),
                                            start=(ky == 0 and kx == 0),
                                            stop=False, perf_mode=DR)
                            for kx in range(3):
                                dx = kx - 1
                                for p in range(NP):
                                    nc.tensor.matmul(
                                        apv(pps[p][:], 0, [[512, 128], [256, 2], [1, N4]]),
                                        W[f'{nm}bd{mt}{kx}'],
                                        apv(ubb[:], (2 * p) * S4 + B4 - G4 + dx,
                                            [[PS4bb, 128], [G4, 2], [S4, 2], [1, N4]]),
                                        start=False, stop=False, perf_mode=DR)
                            for kx in range(3):
                                dx = kx - 1
                                for p in range(NP):
                                    # j0 zero row (reads ky=1 row), j1 real ky=2
                                    nc.tensor.matmul(
                                        apv(pps[p][:], 0, [[512, 128], [256, 2], [1, N4]]),
                                        W[f'{nm}bs{mt}{kx}'],
                                        apv(ubb[:], (2 * p) * S4 + B4 + dx,
                                            [[PS4bb, 128], [G4, 2], [S4, 2], [1, N4]]),
                                        start=False, stop=(kx == 2), perf_mode=DR)
                            for p in range(NP):
                                for q in range(2):
                                    sink(mt, i0 + 2 * p + q,
                                         apv(pps[p][:], q * 256, [[512, 128], [1, N4]]))

                    def sink4(mt, i, p4):
                        i2 = i - i0
                        srcp = apv(p4, 0, [[512, 128], [G4, 13], [1, 13]])
                        if mt < 2:
                            dstu = apv(u4ab[:], i2 * 2 * S4 + mt * S4 + B4,
                                       [[PS4ab, 128], [G4, 13], [1, 13]])
                        else:
                            dstu = apv(u4bb[:], i2 * S4 + B4,
                                       [[PS4bb, 128], [G4, 13], [1, 13]])
                        last_out['DVE'] = nc.vector.tensor_scalar(
                            dstu, srcp, W['thr4u'][:, mt:mt + 1],
                            None, mybir.AluOpType.is_ge)

                    conv45blk('w4', 3, u3ab, u3bb, sink4)
                    if dbg:
                        for i in blk:
                            cba = (i - i0) * 2 * S4
                            cbb = (i - i0) * S4
                            odma(dbg_outs['d_u4a'][i],
                                 apv(u4ab[:], cba, [[PS4ab, 128], [S4, 2], [1, S4]]))
                            odma(dbg_outs['d_u4b'][i],
                                 apv(u4bb[:], cbb, [[PS4bb, 128], [1, S4]]))

                    def sink5(mt, i, p5):
                        c5f = stp.tile([128, N4], mybir.dt.float32, tag="c5f")
                        last_out['ACT'] = nc.scalar.copy(c5f[:], apv(p5, 0, [[512, 128], [1, N4]]))
                        pm5 = stp.tile([128, 13 * 6], mybir.dt.float32, tag="pm5")
                        d = [[N4, 128], [G4, 13], [2, 6]]
                        o = [[13 * 6, 128], [6, 13], [1, 6]]
                        nc.vector.tensor_max(apv(pm5[:], 0, o), apv(c5f[:], 0, d), apv(c5f[:], 1, d))
                        nc.vector.tensor_max(apv(pm5[:], 0, o), apv(pm5[:], 0, o), apv(c5f[:], 2, d))
                        po5 = stp.tile([128, 36], mybir.dt.float32, tag="po5")
                        d2 = [[13 * 6, 128], [12, 6], [1, 6]]
                        o2 = [[36, 128], [6, 6], [1, 6]]
                        nc.vector.tensor_max(apv(po5[:], 0, o2), apv(pm5[:], 0, d2), apv(pm5[:], 6, d2))
                        nc.vector.tensor_max(apv(po5[:], 0, o2), apv(po5[:], 0, o2), apv(pm5[:], 12, d2))
                        hh = i // Bh
                        il = i % Bh
                        h5 = nc.vector.tensor_scalar(
                            apv(t5h[hh][:], mt * 36 * Bh + il,
                                [[2 * 36 * Bh, 128], [Bh, 36]]),
                            po5[:], W['thr5u'][:, mt:mt + 1], None,
                            mybir.AluOpType.is_ge)
                        last_out['DVE'] = h5
                        if i == B - 1:
                            tail_extra.append(h5)

                    conv45blk('w5', 2, u4ab, u4bb, sink5)

            if dbg:
                odma(dbg_outs['d_t5'][:],
                     t5[:].rearrange("p (a b c) -> p a b c", a=2, b=36)
                     if Bp == B else
                     apv(t5[:], 0, [[2 * 36 * Bp, 128], [36 * Bp, 2], [Bp, 36], [1, B]]))

            colp_cm.__exit__(None, None, None)

            # ===== fc phase: tensor-parallel over output features =====
            # conv stays data-parallel (16 img/core); each core then computes a
            # 512-feature slice of fc1/fc2 for ALL 128 images (full-width PE),
            # holding only its 7.5MB weight slice. t5/t6 are exchanged via
            # zero-slotted AllReduce; fc3 partials are summed by ReduceScatter,
            # which hands each core exactly its own 16-image output block.
            with tc.tile_pool(name="fcact", bufs=1) as fca, \
                 tc.tile_pool(name="fcs", bufs=8) as fcs:

                # t5 -> zero-padded slots (slot k live only on core k)
                for k in range(NCORES):
                    nc.vector.tensor_scalar(
                        apv(zb5[:], 16 * k,
                            [[9216, 128], [4608, 2], [128, 36], [1, 16]]),
                        apv(t5[:], 0, [[2 * 36 * Bp, 128], [36 * Bp, 2],
                                       [Bp, 36], [1, 16]]),
                        W['cmask'][:, k:k + 1], None, mybir.AluOpType.mult)
                ldma(ag5_in[:], zb5[:])
                nc.gpsimd.collective_compute(
                    "AllReduce", mybir.AluOpType.add,
                    ins=[ag5_in[:]], outs=[ag5_out[:]], replica_groups=RG)
                t5g = fca.tile([128, 9216], mybir.dt.float8e4, tag="t5g")
                ldma(t5g[:], ag5_out[:])

                # fc1: activations stationary (M = 128 images), weights moving
                with tc.tile_pool(name="psf1", bufs=1, space="PSUM") as psf, \
                     tc.tile_pool(name="pst1", bufs=2, space="PSUM") as pst:
                    pf1 = psf.tile([128, 512], mybir.dt.float32, tag="pf1")
                    for s in range(36):
                        wt = fcs.tile([128, 2, 512], mybir.dt.float8e4, tag="w6s")
                        ldma(wt[:], ins['w6tp'][s])
                        nc.tensor.matmul(
                            pf1[:],
                            apv(t5g[:], s * 128, [[9216, 128], [4608, 2], [1, 128]]),
                            wt[:], start=(s == 0), stop=False, perf_mode=DR)
                    nc.tensor.matmul(pf1[:], t_ones[:], W['thr6'][:],
                                     start=False, stop=True)
                    t6b = fca.tile([128, 512], mybir.dt.float16, tag="t6b")
                    last_out['DVE'] = nc.vector.tensor_scalar(
                        t6b[:], pf1[:], 0.0, None, mybir.AluOpType.is_ge)

                    # transpose [img, feat] -> [feat, img], pack to zb6 slots
                    ptr = pst.tile([128, 1024], mybir.dt.float16, tag="ptr")
                    for ch in range(4):
                        nc.tensor.transpose(ptr[:, ch * 256:ch * 256 + 128],
                                            t6b[:, ch * 128:(ch + 1) * 128],
                                            W['id128'])
                    st6 = fca.tile([128, 512], mybir.dt.float8e4, tag="st6")
                    last_out['ACT'] = nc.scalar.copy(
                        apv(st6[:], 0, [[512, 128], [128, 4], [1, 128]]),
                        apv(ptr[:], 0, [[1024, 128], [256, 4], [1, 128]]))
                    for k in range(NCORES):
                        nc.vector.tensor_scalar(
                            apv(zb6[:], 512 * k, [[4096, 128], [1, 512]]),
                            st6[:], W['cmask'][:, k:k + 1], None,
                            mybir.AluOpType.mult)
                    ldma(ag6_in[:], zb6[:])
                    nc.gpsimd.collective_compute(
                        "AllReduce", mybir.AluOpType.add,
                        ins=[ag6_in[:]], outs=[ag6_out[:]], replica_groups=RG)
                    t6g = fca.tile([128, 4096], mybir.dt.float8e4, tag="t6g")
                    ldma(t6g[:], ag6_out[:])

                    # fc2 (+ folded bn7 offset row), relu; sc7 lives in w8sp
                    pf2 = psf.tile([128, 512], mybir.dt.float32, tag="pf2")
                    for kc in range(16):
                        wt = fcs.tile([128, 2, 512], mybir.dt.float8e4, tag="w7s")
                        ldma(wt[:], ins['w7tp'][kc])
                        nc.tensor.matmul(
                            pf2[:],
                            apv(t6g[:], kc * 256, [[4096, 128], [128, 2], [1, 128]]),
                            wt[:], start=(kc == 0), stop=False, perf_mode=DR)
                    nc.tensor.matmul(pf2[:], t_ones[:], W['crow'][:],
                                     start=False, stop=True)
                    y7 = fca.tile([128, 512], mybir.dt.float16, tag="y7")
                    last_out['ACT'] = nc.scalar.activation(
                        y7[:], pf2[:], mybir.ActivationFunctionType.Relu)

                    ptr2 = pst.tile([128, 1024], mybir.dt.float16, tag="ptr2")
                    for ch in range(4):
                        nc.tensor.transpose(ptr2[:, ch * 256:ch * 256 + 128],
                                            y7[:, ch * 128:(ch + 1) * 128],
                                            W['id128'])
                    y7t = fca.tile([128, 512], mybir.dt.float16, tag="y7t")
                    last_out['ACT'] = nc.scalar.copy(
                        apv(y7t[:], 0, [[512, 128], [128, 4], [1, 128]]),
                        apv(ptr2[:], 0, [[1024, 128], [256, 4], [1, 128]]))

                    # fc3 partial over this core's 512 channels, all 128 images
                    pf3af = psf.tile([128, 512], mybir.dt.float32, tag="pf3a")
                    pf3a = pf3af[:, 0:500]
                    pf3bf = psf.tile([128, 512], mybir.dt.float32, tag="pf3b")
                    pf3b = pf3bf[:, 0:500]
                    wt8 = fca.tile([128, 4000], mybir.dt.float16, tag="w8s")
                    ldma(wt8[:],
                         apv(ins['w8sp'][0], 0,
                             [[1000, 128], [128 * 1000, 4], [1, 1000]]))
                    for ch in range(4):
                        lhs = y7t[:, ch * 128:(ch + 1) * 128]
                        nc.tensor.matmul(pf3a, lhs, wt8[:, ch * 1000:ch * 1000 + 500],
                                         start=(ch == 0), stop=False)
                        nc.tensor.matmul(pf3b, lhs, wt8[:, ch * 1000 + 500:(ch + 1) * 1000],
                                         start=(ch == 0), stop=False)
                    nc.tensor.matmul(pf3a, t_ones[:], W['b8d'][:, 0:500],
                                     start=False, stop=True)
                    nc.tensor.matmul(pf3b, t_ones[:], W['b8d'][:, 500:1000],
                                     start=False, stop=True)

                    of = fca.tile([128, 1000], mybir.dt.float32, tag="of")
                    nc.vector.tensor_copy(of[:, 0:500], pf3a)
                    nc.vector.tensor_copy(of[:, 500:1000], pf3b)
                    ldma(rs_in[:], of[:])
                    nc.gpsimd.collective_compute(
                        "ReduceScatter", mybir.AluOpType.add,
                        ins=[rs_in[:]], outs=[rs_out[:]], replica_groups=RG)
                    oo = fca.tile([B, 1000], mybir.dt.float32, tag="oo")
                    h_of = nc.sync.dma_start(oo[:], rs_out[:])
                    dma_handles.append(h_of)
                    dma_handles.append(nc.sync.dma_start(out[:], oo[:]))

            # ---------- tail-sync for the final drain ----------
            for h in dma_handles[-40:] + tail_extra:
                n = nc.sync.nop(nofuse=True)
                add_dep_helper(n.ins, h.ins, reason="tail drain sync")

    legalize_waits(nc)
    return nc


def legalize_waits(nc):
    """Split multi-wait sync lists into single-wait same-engine NOPs.

    TPB instructions (compute, NOP, drain, DMA pseudo-ops) accept one
    sync-wait command in this walrus; extra waits are moved onto freshly
    inserted NOPs placed directly before the instruction in its basic block
    (same engine stream).
    """
    f = nc.m.functions[0]
    ctr = 0
    for blk in f.blocks:
        new = []
        for inst in blk.instructions:
            si = inst.sync_info
            if si is not None and inst.engine is not None:
                waits = list(si.on_wait)
                if len(waits) > 1:
                    for w in waits[:-1]:
                        ctr += 1
                        n = mybir.InstNoOp(name=f"I-wfix{ctr}", ins=[], outs=[])
                        n.engine = inst.engine
                        n.sync_info = bass_rust.SyncInfo(on_wait=[w], on_update=[])
                        new.append(n)
                    inst.sync_info = bass_rust.SyncInfo(
                        on_wait=[waits[-1]], on_update=list(si.on_update))
            new.append(inst)
        blk.instructions = new
    return ctr


# ======================= entry point =======================

def make_in_maps(P, B):
    in_maps = []
    for c in range(NCORES):
        f0, f1 = 512 * c, 512 * (c + 1)
        cm = np.zeros((128, 8), np.float32)
        cm[:, c] = 1.0
        percore = {
            'wpf16': np.concatenate(
                [P['thr6_full'][:, f0:f1], P['crow_full'][:, f0:f1],
                 P['b8d']], axis=1).astype(np.float16),
            'cmask': cm,
            'w6tp': P['w6t_full'][:, :, :, f0:f1],
            'w7tp': P['w7t_full'][:, :, :, f0:f1],
            'w8sp': P['w8s_full'][4 * c:4 * (c + 1)],
        }
        m = {}
        for name, shp, dt, mode in IN_SPECS:
            if mode == 'core':
                a = percore[name]
            else:
                a = P[name]
                if mode == 'img':
                    a = a[c * B:(c + 1) * B]
            ref_shape = (B,) + tuple(shp) if mode == 'img' else tuple(shp)
            assert tuple(a.shape) == ref_shape, (name, a.shape, ref_shape)
            m[name] = np.ascontiguousarray(a)
        in_maps.append(m)
    return in_maps


def kernel(**inputs) -> np.ndarray:
    P = prep_host(inputs)
    B = P['colhl'].shape[0] // NCORES
    nc = build_module(B, dbg=False)
    in_maps = make_in_maps(P, B)
    res = run_bass_kernel_spmd(nc, in_maps, core_ids=list(range(NCORES)))
    outs = [res.results[c]['out'] for c in range(NCORES)]
    return np.concatenate(outs, axis=0).astype(np.float32)

